# revision 1
# baseline (speedup 1.0000x reference)
"""Distributed GCN classifier kernel for 8 Trainium2 NeuronCores (Bass/Tile).

Strategy (dest-node row sharding, per spec sharding_hint):
- Core c owns dest nodes [c*NLOC, (c+1)*NLOC); within each graph nodes are
  permuted by in-degree so per-dest-tile edge counts stay balanced.
- Per dest tile, edges are gathered edge-major with dma_gather (int16
  indices force a lo/hi table split at N/2) into [128 x C x F] SBUF tiles;
  the segment-sum is a PE matmul with per-chunk one-hot selector matrices
  S[p, d] = (d == dest_local[p]) built by one fused DVE tensor_scalar
  (is_equal) per 128-edge chunk, accumulating in PSUM.
- Layer 1 gathers rows of (dinv*X) from the input table (W1 applied after
  aggregation via PE transpose + matmul); layer 2 gathers rows of
  Y2 = dinv*(h1@W2.T) from an AllGathered internal DRAM table.
- Normalization: v = val*dinv[row]*dinv[col]; dinv[col] folded into tables,
  dinv[row] (+ const val) folded into the PSUM->SBUF copy scale; general
  (non-const) val is folded into S instead (dual-op tensor_scalar).
- LayerNorm per dest tile on the free dim; pooling via static per-graph
  slices of the PE-transposed h [feat x node] block; classifier on-core.

kernel(**inputs) takes the full unsharded inputs and returns the full
[B, 2] logits; sharding/unsharding happens on host inside this function.
"""
import sys

import numpy as np

sys.path.insert(0, "/opt/trn_rl_repo")

from contextlib import ExitStack

import concourse.bass as bass
import concourse.bacc as bacc
import concourse.tile as tile
from concourse import mybir
from concourse.bass_utils import run_bass_kernel_spmd
from concourse.masks import make_identity

NCORES = 8
P = 128
F32 = mybir.dt.float32
I16 = mybir.dt.int16
AF = mybir.ActivationFunctionType
ALU = mybir.AluOpType
AX = mybir.AxisListType


# ----------------------------------------------------------------- host prep
def _prep(X, edge_index, edge_val, ptr, W1, W2, Wres, ln_gamma, ln_beta, Wcls,
          b_cls):
    N, DIN = X.shape
    HID = W1.shape[0]
    OUT = Wcls.shape[0]
    E = edge_index.shape[1]
    B = ptr.shape[0] - 1

    row = np.asarray(edge_index[0], dtype=np.int64)
    col = np.asarray(edge_index[1], dtype=np.int64)
    val = np.asarray(edge_val, dtype=np.float32)
    ptr = np.asarray(ptr, dtype=np.int64)

    assert N % (NCORES * P) == 0, (N, NCORES * P)
    NLOC = N // NCORES
    TILES = NLOC // P
    HALF = NLOC * (NCORES // 2)
    assert HALF < 2 ** 15 and N - HALF < 2 ** 15  # int16 gather index range

    deg = np.bincount(row, weights=val.astype(np.float64), minlength=N)
    deg = np.clip(deg, 1e-9, None)
    dinv = (1.0 / np.sqrt(deg)).astype(np.float32)

    val_const = float(val[0]) if E > 0 else 1.0
    val_is_const = bool(np.all(val == val_const))

    seg_len = ptr[1:] - ptr[:-1]
    uniform = (
        B > 0 and N % B == 0
        and bool(np.all(seg_len == N // B))
        and NLOC % (N // B) == 0
    )
    assert uniform, "non-uniform ptr not supported by this build"
    GN = N // B
    GPC = NLOC // GN

    perm = np.empty(N, dtype=np.int64)
    for b in range(B):
        lo, hi = int(ptr[b]), int(ptr[b + 1])
        seg = np.arange(lo, hi)
        order = np.argsort(deg[lo:hi], kind="stable")
        perm[lo:hi] = seg[order]
    invperm = np.empty(N, dtype=np.int64)
    invperm[perm] = np.arange(N)

    # order edges by (permuted dest pos, src-half)
    lp_all = invperm[row]
    is_hi = (col >= HALF).astype(np.int64)
    order_e = np.lexsort((np.arange(E), is_hi, lp_all // P))
    lp_s = lp_all[order_e]
    hi_s = is_hi[order_e]
    col_s = col[order_e]
    val_s = val[order_e]

    g_tile = lp_s // P                      # global tile id (core*TILES + t)
    key = g_tile * 2 + hi_s                 # (global tile, half)
    cnt = np.bincount(key, minlength=NCORES * TILES * 2)
    cnt3 = cnt.reshape(NCORES, TILES, 2)
    C_th = np.ceil(cnt3.max(axis=0) / P).astype(np.int64)   # [TILES, 2]
    C_th = np.maximum(C_th, 1)
    CPT = C_th.sum(axis=1)
    SUMC = int(CPT.sum())
    cumC = np.concatenate([[0], np.cumsum(CPT)])
    CMAX = int(C_th.max())

    # per-edge position within its (core, tile, half) stream
    rank = np.arange(E) - np.searchsorted(key, key)

    dl = np.full((NCORES, P, SUMC), -1.0, dtype=np.float32)
    wslot = np.zeros((NCORES, P, SUMC), dtype=np.float32)
    idx1 = np.zeros((NCORES, P, SUMC * 8), dtype=np.int16)
    idx2 = np.zeros((NCORES, P, SUMC * 8), dtype=np.int16)

    e_core = lp_s // NLOC
    e_t = (lp_s % NLOC) // P
    e_p = rank % P
    e_c = rank // P
    chunk_g = cumC[e_t] + hi_s * C_th[e_t, 0] + e_c
    d_loc = lp_s % P

    dl[e_core, e_p, chunk_g] = d_loc.astype(np.float32)
    wslot[e_core, e_p, chunk_g] = val_s
    # int16 gather indices: stream position i -> [col i//16, partition i%16],
    # replicated across the 8 16-partition groups.
    i1 = np.where(hi_s == 0, col_s, col_s - HALF).astype(np.int16)
    r2 = (col_s // NLOC) * NLOC + (invperm[col_s] % NLOC)
    i2 = np.where(r2 < HALF, r2, r2 - HALF).astype(np.int16)
    colbase = (cumC[e_t] + hi_s * C_th[e_t, 0]) * 8
    icol = colbase + rank // 16
    ipart = rank % 16
    for g in range(8):
        idx1[e_core, 16 * g + ipart, icol] = i1
        idx2[e_core, 16 * g + ipart, icol] = i2

    pg = perm.reshape(NCORES, TILES, P)
    dinv_d = dinv[pg].transpose(0, 2, 1)  # [core, P, TILES]
    # post-scale: const val folds in here; per-edge val goes via S instead
    dinv_c = dinv_d * np.float32(val_const if val_is_const else 1.0)

    X = np.asarray(X, dtype=np.float32)
    xtab = np.ascontiguousarray(X * dinv[:, None])

    iota = np.tile(np.arange(P, dtype=np.float32)[None, :], (P, 1))

    meta = dict(N=N, E=E, DIN=DIN, HID=HID, OUT=OUT, B=B, NLOC=NLOC,
                TILES=TILES, HALF=HALF, GN=GN, GPC=GPC,
                C_th=[(int(a), int(b)) for a, b in C_th], SUMC=SUMC,
                CMAX=CMAX, val_is_const=val_is_const, val_const=val_const,
                ln_trivial=bool(np.all(np.asarray(ln_gamma) == 1.0)
                                and np.all(np.asarray(ln_beta) == 0.0)))

    shared = dict(
        xtab=xtab,
        iota=np.ascontiguousarray(iota),
        w1t=np.ascontiguousarray(np.asarray(W1, np.float32).T),
        w2t=np.ascontiguousarray(np.asarray(W2, np.float32).T),
        wrest=np.ascontiguousarray(np.asarray(Wres, np.float32).T),
        wclst=np.ascontiguousarray(np.asarray(Wcls, np.float32).T),
        bcls=np.ascontiguousarray(np.asarray(b_cls, np.float32)[:, None]),
        gam=np.ascontiguousarray(np.asarray(ln_gamma, np.float32)[None, :]),
        bet=np.ascontiguousarray(np.asarray(ln_beta, np.float32)[None, :]),
    )
    percore = []
    for c in range(NCORES):
        percore.append(dict(
            idx1=np.ascontiguousarray(idx1[c]),
            idx2=np.ascontiguousarray(idx2[c]),
            dl=np.ascontiguousarray(dl[c]),
            wslot=np.ascontiguousarray(wslot[c]),
            dinv_d=np.ascontiguousarray(dinv_d[c]),
            dinv_c=np.ascontiguousarray(dinv_c[c]),
            xt_own=np.ascontiguousarray(X[pg[c].reshape(-1)].T),
        ))
    return meta, shared, percore


# ------------------------------------------------------------- device program
def _build(meta):
    M = meta
    TILES, SUMC, CMAX = M["TILES"], M["SUMC"], M["CMAX"]
    DIN, HID, OUT = M["DIN"], M["HID"], M["OUT"]
    NLOC, HALF = M["NLOC"], M["HALF"]
    C_th = M["C_th"]
    cumC = [0]
    for a, b in C_th:
        cumC.append(cumC[-1] + a + b)
    DCH = DIN // P
    general_val = not M["val_is_const"]

    nc = bacc.Bacc(num_devices=NCORES)

    # ---- DRAM I/O
    xtab_d = nc.dram_tensor("xtab", [M["N"], DIN], F32, kind="ExternalInput")
    xt_own_d = nc.dram_tensor("xt_own", [DIN, NLOC], F32, kind="ExternalInput")
    idx1_d = nc.dram_tensor("idx1", [P, SUMC * 8], I16, kind="ExternalInput")
    idx2_d = nc.dram_tensor("idx2", [P, SUMC * 8], I16, kind="ExternalInput")
    dl_d = nc.dram_tensor("dl", [P, SUMC], F32, kind="ExternalInput")
    iota_d = nc.dram_tensor("iota", [P, P], F32, kind="ExternalInput")
    dinv_d_d = nc.dram_tensor("dinv_d", [P, TILES], F32, kind="ExternalInput")
    dinv_c_d = nc.dram_tensor("dinv_c", [P, TILES], F32, kind="ExternalInput")
    w1t_d = nc.dram_tensor("w1t", [DIN, HID], F32, kind="ExternalInput")
    w2t_d = nc.dram_tensor("w2t", [HID, HID], F32, kind="ExternalInput")
    wrest_d = nc.dram_tensor("wrest", [DIN, HID], F32, kind="ExternalInput")
    wclst_d = nc.dram_tensor("wclst", [2 * HID, OUT], F32, kind="ExternalInput")
    bcls_d = nc.dram_tensor("bcls", [OUT, 1], F32, kind="ExternalInput")
    if general_val:
        wslot_d = nc.dram_tensor("wslot", [P, SUMC], F32, kind="ExternalInput")
    if not M["ln_trivial"]:
        gam_d = nc.dram_tensor("gam", [1, HID], F32, kind="ExternalInput")
        bet_d = nc.dram_tensor("bet", [1, HID], F32, kind="ExternalInput")
    out_d = nc.dram_tensor("logits_t", [OUT, M["GPC"]], F32,
                           kind="ExternalOutput")

    y2own_d = nc.dram_tensor("y2own", [NLOC, HID], F32)
    xres_d = nc.dram_tensor("xres_dram", [NLOC, HID], F32)
    y2full_d = nc.dram_tensor("y2full", [NCORES * NLOC, HID], F32,
                              addr_space="Shared")

    with tile.TileContext(nc) as tc, ExitStack() as ctx:
        cpool = ctx.enter_context(tc.tile_pool(name="consts", bufs=1))
        gpool = ctx.enter_context(tc.tile_pool(name="gather", bufs=3))
        spool = ctx.enter_context(tc.tile_pool(name="small", bufs=4))
        Spool = ctx.enter_context(tc.tile_pool(name="sel", bufs=6))
        ppool = ctx.enter_context(tc.tile_pool(name="psum", bufs=2, space="PSUM"))
        blkpool = ctx.enter_context(tc.tile_pool(name="blocks", bufs=1))

        # ---- constants / resident blocks
        ident = cpool.tile([P, P], F32)
        make_identity(nc, ident[:])
        eps_sb = cpool.tile([P, 1], F32, tag="eps")
        nc.vector.memset(eps_sb[:], float(HID * 1e-5))
        iota_sb = cpool.tile([P, P], F32, tag="iota")
        nc.sync.dma_start(iota_sb[:], iota_d[:])

        idx1_sb = cpool.tile([P, SUMC * 8], I16, tag="idx1")
        nc.sync.dma_start(idx1_sb[:], idx1_d[:])
        idx2_sb = cpool.tile([P, SUMC * 8], I16, tag="idx2")
        nc.sync.dma_start(idx2_sb[:], idx2_d[:])
        dl_sb = cpool.tile([P, SUMC], F32, tag="dl")
        nc.sync.dma_start(dl_sb[:], dl_d[:])
        dinv_sb = cpool.tile([P, TILES], F32, tag="dinv")
        nc.sync.dma_start(dinv_sb[:], dinv_d_d[:])
        dinvc_sb = cpool.tile([P, TILES], F32, tag="dinvc")
        nc.sync.dma_start(dinvc_sb[:], dinv_c_d[:])
        if general_val:
            wslot_sb = cpool.tile([P, SUMC], F32, tag="wslot")
            nc.sync.dma_start(wslot_sb[:], wslot_d[:])

        w1t_sb = [cpool.tile([P, HID], F32, tag=f"w1t{i}", name=f"w1t_sb{i}")
                  for i in range(DCH)]
        for i in range(DCH):
            nc.sync.dma_start(w1t_sb[i][:], w1t_d[i * P:(i + 1) * P, :])
        w2t_sb = cpool.tile([HID, HID], F32, tag="w2t")
        nc.sync.dma_start(w2t_sb[:], w2t_d[:])
        wrest_sb = [cpool.tile([P, HID], F32, tag=f"wrest{i}", name=f"wrest_sb{i}")
                    for i in range(DCH)]
        for i in range(DCH):
            nc.sync.dma_start(wrest_sb[i][:], wrest_d[i * P:(i + 1) * P, :])
        wclst_sb = [cpool.tile([P, OUT], F32, tag=f"wclst{i}", name=f"wclst_sb{i}")
                    for i in range(2)]
        for i in range(2):
            nc.sync.dma_start(wclst_sb[i][:], wclst_d[i * HID:(i + 1) * HID, :])
        bcls_sb = cpool.tile([OUT, 1], F32, tag="bcls")
        nc.sync.dma_start(bcls_sb[:], bcls_d[:])

        if not M["ln_trivial"]:
            grow = cpool.tile([1, HID], F32, tag="grow")
            nc.sync.dma_start(grow[:], gam_d[:])
            brow = cpool.tile([1, HID], F32, tag="brow")
            nc.sync.dma_start(brow[:], bet_d[:])
            ones1 = cpool.tile([1, P], F32, tag="ones1")
            nc.vector.memset(ones1[:], 1.0)
            gb_ps = ppool.tile([P, HID], F32, tag="mm")
            nc.tensor.matmul(gb_ps[:], lhsT=ones1[:], rhs=grow[:],
                             start=True, stop=True)
            gam_sb = cpool.tile([P, HID], F32, tag="gam_sb")
            nc.scalar.copy(gam_sb[:], gb_ps[:])
            bb_ps = ppool.tile([P, HID], F32, tag="mm")
            nc.tensor.matmul(bb_ps[:], lhsT=ones1[:], rhs=brow[:],
                             start=True, stop=True)
            bet_sb = cpool.tile([P, HID], F32, tag="bet_sb")
            nc.scalar.copy(bet_sb[:], bb_ps[:])

        h1T = blkpool.tile([HID, NLOC], F32, tag="h1T")
        hT = blkpool.tile([HID, NLOC], F32, tag="h1T", name="hT")

        # ---- Xres = X_own @ Wres.T (lhsT = Xt_own chunks), spilled to DRAM
        for t in range(TILES):
            xps = ppool.tile([P, HID], F32, tag="mm")
            for i in range(DCH):
                xt_sb = spool.tile([P, P], F32, tag="xt_chunk")
                nc.sync.dma_start(
                    xt_sb[:], xt_own_d[i * P:(i + 1) * P, t * P:(t + 1) * P])
                nc.tensor.matmul(xps[:], lhsT=xt_sb[:], rhs=wrest_sb[i][:],
                                 start=(i == 0), stop=(i == DCH - 1))
            xres_sb = spool.tile([P, HID], F32, tag="xres_sb")
            nc.scalar.copy(xres_sb[:], xps[:])
            nc.sync.dma_start(xres_d[t * P:(t + 1) * P, :], xres_sb[:])

        def spmm_tile(t, idx_sb, tab_lo, tab_hi, F, agg_ps):
            """Gather both halves of tile t and accumulate the one-hot
            matmul segment-sum into agg_ps [P, F]."""
            n_ch = C_th[t][0] + C_th[t][1]
            done = 0
            for half in range(2):
                C = C_th[t][half]
                cb = cumC[t] + (C_th[t][0] if half else 0)
                g = gpool.tile([P, CMAX * DIN], F32, tag="g", name="gt")
                gv = g[:, :C * F].rearrange("p (c f) -> p c f", f=F)
                nc.gpsimd.dma_gather(
                    gv, tab_hi if half else tab_lo,
                    idx_sb[:, cb * 8:(cb + C) * 8],
                    C * P, C * P, F, single_packet=False)
                for c in range(C):
                    S = Spool.tile([P, P], F32, tag="S", name="St")
                    if general_val:
                        nc.vector.tensor_scalar(
                            out=S[:], in0=iota_sb[:],
                            scalar1=dl_sb[:, cb + c:cb + c + 1],
                            scalar2=wslot_sb[:, cb + c:cb + c + 1],
                            op0=ALU.is_equal, op1=ALU.mult)
                    else:
                        nc.vector.tensor_scalar(
                            out=S[:], in0=iota_sb[:],
                            scalar1=dl_sb[:, cb + c:cb + c + 1],
                            scalar2=None, op0=ALU.is_equal)
                    nc.tensor.matmul(
                        agg_ps[:], lhsT=S[:], rhs=g[:, c * F:(c + 1) * F],
                        start=(done == 0), stop=(done == n_ch - 1))
                    done += 1

        # ---- layer 1: agg = A_w @ xtab ; h1T = relu(W1 @ (dinv_c*agg).T)
        for t in range(TILES):
            agg_ps = ppool.tile([P, DIN], F32, tag="agg")
            spmm_tile(t, idx1_sb, xtab_d[:HALF, :], xtab_d[HALF:, :], DIN,
                      agg_ps)
            agg = spool.tile([P, DIN], F32, tag="agg_sb")
            nc.scalar.activation(agg[:], agg_ps[:], AF.Copy,
                                 scale=dinvc_sb[:, t:t + 1])
            aggT = []
            for i in range(DCH):
                tps = ppool.tile([P, P], F32, tag="tr")
                nc.tensor.transpose(tps[:], agg[:, i * P:(i + 1) * P], ident[:])
                aT = spool.tile([P, P], F32, tag=f"aggT{i}", name=f"aggT_{i}")
                nc.scalar.copy(aT[:], tps[:])
                aggT.append(aT)
            h1ps = ppool.tile([P, P], F32, tag="mm")
            for i in range(DCH):
                nc.tensor.matmul(h1ps[:], lhsT=w1t_sb[i][:], rhs=aggT[i][:],
                                 start=(i == 0), stop=(i == DCH - 1))
            nc.scalar.activation(h1T[:, t * P:(t + 1) * P], h1ps[:], AF.Relu)

        # ---- Y2 = dinv * (h1 @ W2.T); write own shard; AllGather
        for t in range(TILES):
            yps = ppool.tile([P, HID], F32, tag="mm")
            nc.tensor.matmul(yps[:], lhsT=h1T[:, t * P:(t + 1) * P],
                             rhs=w2t_sb[:], start=True, stop=True)
            y2sb = spool.tile([P, HID], F32, tag="y2_sb")
            nc.scalar.activation(y2sb[:], yps[:], AF.Copy,
                                 scale=dinv_sb[:, t:t + 1])
            nc.sync.dma_start(y2own_d[t * P:(t + 1) * P, :], y2sb[:])
        nc.gpsimd.collective_compute(
            "AllGather", ALU.bypass,
            replica_groups=[list(range(NCORES))],
            ins=[y2own_d[:]], outs=[y2full_d[:]])

        # ---- layer 2 + LN + transpose into hT
        for t in range(TILES):
            agg_ps = ppool.tile([P, HID], F32, tag="agg")
            spmm_tile(t, idx2_sb, y2full_d[:HALF, :], y2full_d[HALF:, :], HID,
                      agg_ps)
            h2 = spool.tile([P, HID], F32, tag="h2")
            nc.scalar.activation(h2[:], agg_ps[:], AF.Relu,
                                 scale=dinvc_sb[:, t:t + 1])
            xres_t = spool.tile([P, HID], F32, tag="xres_t")
            nc.sync.dma_start(xres_t[:], xres_d[t * P:(t + 1) * P, :])
            nc.vector.tensor_tensor(
                out=h2[:], in0=h2[:], in1=xres_t[:], op=ALU.add)
            # LayerNorm: rstd' = 1/sqrt(ss + HID*eps); hn = (x-mu)*rstd'*sqrt(HID)
            mu = spool.tile([P, 1], F32, tag="mu")
            nc.vector.tensor_reduce(mu[:], h2[:], axis=AX.X, op=ALU.add)
            nc.vector.tensor_scalar_mul(mu[:], mu[:], 1.0 / HID)
            nc.vector.tensor_scalar_sub(h2[:], h2[:], mu[:])
            sq = spool.tile([P, HID], F32, tag="sq")
            nc.vector.tensor_tensor(out=sq[:], in0=h2[:], in1=h2[:],
                                    op=ALU.mult)
            var = spool.tile([P, 1], F32, tag="var")
            nc.vector.tensor_reduce(var[:], sq[:], axis=AX.X, op=ALU.add)
            std = spool.tile([P, 1], F32, tag="std")
            nc.scalar.activation(std[:], var[:], AF.Sqrt,
                                 bias=eps_sb[:], scale=1.0)
            rstd = spool.tile([P, 1], F32, tag="rstd")
            nc.vector.reciprocal(rstd[:], std[:])
            nc.vector.tensor_scalar(
                out=h2[:], in0=h2[:], scalar1=rstd[:],
                scalar2=float(np.sqrt(HID)), op0=ALU.mult, op1=ALU.mult)
            if not M["ln_trivial"]:
                nc.vector.tensor_tensor(out=h2[:], in0=h2[:], in1=gam_sb[:],
                                        op=ALU.mult)
                nc.vector.tensor_tensor(out=h2[:], in0=h2[:], in1=bet_sb[:],
                                        op=ALU.add)
            tps = ppool.tile([P, P], F32, tag="tr")
            nc.tensor.transpose(tps[:], h2[:], ident[:])
            nc.scalar.copy(hT[:, t * P:(t + 1) * P], tps[:])

        # ---- pooling + classifier
        GN, GPC = M["GN"], M["GPC"]
        Hcat = spool.tile([P, 2 * GPC], F32, tag="Hcat")  # [f, mean|max x g]
        for g_ in range(GPC):
            nc.vector.tensor_reduce(
                Hcat[:, g_:g_ + 1], hT[:, g_ * GN:(g_ + 1) * GN],
                axis=AX.X, op=ALU.add)
            nc.vector.tensor_reduce(
                Hcat[:, GPC + g_:GPC + g_ + 1], hT[:, g_ * GN:(g_ + 1) * GN],
                axis=AX.X, op=ALU.max)
        nc.vector.tensor_scalar_mul(Hcat[:, :GPC], Hcat[:, :GPC], 1.0 / GN)
        ops = ppool.tile([OUT, GPC], F32, tag="mm")
        nc.tensor.matmul(ops[:], lhsT=wclst_sb[0][:], rhs=Hcat[:, :GPC],
                         start=True, stop=False)
        nc.tensor.matmul(ops[:], lhsT=wclst_sb[1][:], rhs=Hcat[:, GPC:],
                         start=False, stop=True)
        osb = spool.tile([OUT, GPC], F32, tag="out_sb")
        nc.vector.tensor_copy(osb[:], ops[:])
        nc.vector.tensor_scalar_add(osb[:], osb[:], bcls_sb[:])
        nc.sync.dma_start(out_d[:], osb[:])

    nc.compile()
    return nc


def _make_in_maps(meta, shared, percore):
    in_maps = []
    for c in range(NCORES):
        m = dict(shared)
        if meta["ln_trivial"]:
            m.pop("gam"), m.pop("bet")
        keys = ["idx1", "idx2", "dl", "dinv_d", "dinv_c", "xt_own"]
        if not meta["val_is_const"]:
            keys.append("wslot")
        for k in keys:
            m[k] = percore[c][k]
        in_maps.append(m)
    return in_maps


_CACHE = {}


def kernel(**inputs):
    meta, shared, percore = _prep(**inputs)
    key = (meta["N"], meta["E"], meta["DIN"], meta["HID"], meta["OUT"],
           meta["B"], tuple(meta["C_th"]), meta["val_is_const"],
           meta["ln_trivial"])
    if key not in _CACHE:
        _CACHE[key] = _build(meta)
    nc = _CACHE[key]

    in_maps = _make_in_maps(meta, shared, percore)
    res = run_bass_kernel_spmd(nc, in_maps, list(range(NCORES)))
    outs = [np.asarray(res.results[c]["logits_t"]).T for c in range(NCORES)]
    return np.ascontiguousarray(np.concatenate(outs, axis=0), dtype=np.float32)



# revision 6
# speedup vs baseline: 1.5815x; 1.5815x over previous
"""Distributed GCN classifier kernel for 8 Trainium2 NeuronCores (Bass/Tile).

v2 strategy (dest-node row sharding; heavy host-side LAYOUT prep, all model
math on device):
- Core c owns dest rows [c*NLOC, (c+1)*NLOC) of the degree-permuted node
  order (permuted within each graph so per-tile edge counts balance).
- Layer-1 SpMM: the edge-major source-feature stream G1 (rows of dinv*X per
  edge, dest-sorted/chunked) and the per-chunk one-hot scatter matrices S1
  (weighted by val*dinv_dest^2) are packed on the HOST (pure layout /
  elementwise scaling, no host GEMMs) and STREAMED as bf16 - zero per-edge
  DMA descriptors and zero DVE one-hot builds on device.  Aggregation is
  aggT[f,d] += G1_chunk[e,f].T-matmul, giving feature-major tiles directly.
- h1T = relu(W1T-matmul of aggT); Y2T = W2T-matmul; per-tile PE transpose
  writes the bf16 Y2 table; AllGather; layer 2 gathers Y2 rows per edge with
  dma_gather (the only per-edge descriptor work left), scattered with
  host-packed S2 (weighted val*dinv_dest).
- h2T = relu(agg2T) + XresT (on-device Wres matmuls); LayerNorm done in
  feature-major space (column stats via ones-matmul + PE broadcast rows);
  pooling accumulates per-graph mean/max incrementally; classifier on-core.

kernel(**inputs) takes the full unsharded inputs and returns the full
[B, 2] logits; sharding/unsharding happens on host inside this function.
"""
import sys

import numpy as np

sys.path.insert(0, "/opt/trn_rl_repo")

from contextlib import ExitStack

import ml_dtypes

import concourse.bass as bass
import concourse.bacc as bacc
import concourse.tile as tile
from concourse import mybir
from concourse.bass_utils import run_bass_kernel_spmd
from concourse.masks import make_identity

NCORES = 8
P = 128
F32 = mybir.dt.float32
BF16 = mybir.dt.bfloat16
I16 = mybir.dt.int16
AF = mybir.ActivationFunctionType
ALU = mybir.AluOpType
AX = mybir.AxisListType
NPBF = ml_dtypes.bfloat16


# ----------------------------------------------------------------- host prep
def _prep(X, edge_index, edge_val, ptr, W1, W2, Wres, ln_gamma, ln_beta, Wcls,
          b_cls):
    N, DIN = X.shape
    HID = W1.shape[0]
    OUT = Wcls.shape[0]
    E = edge_index.shape[1]
    B = ptr.shape[0] - 1

    row = np.asarray(edge_index[0], dtype=np.int64)
    col = np.asarray(edge_index[1], dtype=np.int64)
    val = np.asarray(edge_val, dtype=np.float64)
    ptr = np.asarray(ptr, dtype=np.int64)

    assert N % (NCORES * P) == 0, (N, NCORES * P)
    NLOC = N // NCORES
    TILES = NLOC // P
    HALF = NLOC * (NCORES // 2)
    assert HALF < 2 ** 15 and N - HALF < 2 ** 15  # int16 gather index range

    deg = np.bincount(row, weights=val, minlength=N)
    deg = np.clip(deg, 1e-9, None)
    dinv = 1.0 / np.sqrt(deg)

    seg_len = ptr[1:] - ptr[:-1]
    uniform = (
        B > 0 and N % B == 0
        and bool(np.all(seg_len == N // B))
        and NLOC % (N // B) == 0
    )
    assert uniform, "non-uniform ptr not supported by this build"
    GN = N // B
    GPC = NLOC // GN

    # permute nodes within each graph by degree so per-tile edge counts
    # stay balanced across cores and tiles
    perm = np.empty(N, dtype=np.int64)
    for b in range(B):
        lo, hi = int(ptr[b]), int(ptr[b + 1])
        seg = np.arange(lo, hi)
        order = np.argsort(deg[lo:hi], kind="stable")
        perm[lo:hi] = seg[order]
    invperm = np.empty(N, dtype=np.int64)
    invperm[perm] = np.arange(N)

    lp_all = invperm[row]          # dest position of each edge
    xtab = (np.asarray(X, np.float64) * dinv[:, None])  # dinv_col-scaled rows

    # ---------------- layer-1 stream (no half split needed; no gather)
    order1 = np.lexsort((np.arange(E), lp_all // P))
    lp1 = lp_all[order1]
    col1 = col[order1]
    w1e = (val * dinv[row] ** 2)[order1]       # val * dinv_dest^2

    gt1 = lp1 // P                              # global tile = core*TILES + t
    cnt1 = np.bincount(gt1, minlength=NCORES * TILES).reshape(NCORES, TILES)
    C1 = np.maximum(np.ceil(cnt1.max(axis=0) / P).astype(np.int64), 1)
    SUMC1 = int(C1.sum())
    cum1 = np.concatenate([[0], np.cumsum(C1)])
    C1MAX = int(C1.max())

    rank1 = np.arange(E) - np.searchsorted(gt1, gt1)
    e_core1 = lp1 // NLOC
    e_t1 = (lp1 % NLOC) // P
    e_p1 = rank1 % P
    e_c1 = cum1[e_t1] + rank1 // P
    d_loc1 = lp1 % P

    G1 = np.zeros((NCORES, P, SUMC1 * DIN), dtype=NPBF)
    S1 = np.zeros((NCORES, P, SUMC1 * P), dtype=NPBF)
    G1_rows = xtab[col1].astype(NPBF)           # [E, DIN]
    # scatter rows into [core, partition, chunk-block]
    ccol = (e_c1[:, None] * DIN + np.arange(DIN)[None, :])
    G1[e_core1[:, None], e_p1[:, None], ccol] = G1_rows
    S1[e_core1, e_p1, e_c1 * P + d_loc1] = w1e.astype(NPBF)

    # ---------------- layer-2 gather tables (lo/hi half split for int16)
    r2_all = invperm[col]                       # source position of each edge
    hi2 = (r2_all >= HALF).astype(np.int64)
    order2 = np.lexsort((np.arange(E), hi2, lp_all // P))
    lp2 = lp_all[order2]
    r2 = r2_all[order2]
    h2s = hi2[order2]
    w2e = (val * dinv[row])[order2]             # val * dinv_dest

    gt2 = lp2 // P
    key2 = gt2 * 2 + h2s
    cnt2 = np.bincount(key2, minlength=NCORES * TILES * 2)
    cnt2 = cnt2.reshape(NCORES, TILES, 2)
    C2 = np.maximum(np.ceil(cnt2.max(axis=0) / P).astype(np.int64), 1)
    CT2 = C2.sum(axis=1)
    SUMC2 = int(CT2.sum())
    cum2 = np.concatenate([[0], np.cumsum(CT2)])
    C2TOTMAX = int(CT2.max())

    rank2 = np.arange(E) - np.searchsorted(key2, key2)
    e_core2 = lp2 // NLOC
    e_t2 = (lp2 % NLOC) // P
    e_p2 = rank2 % P
    chunk2 = cum2[e_t2] + h2s * C2[e_t2, 0] + rank2 // P
    d_loc2 = lp2 % P

    S2 = np.zeros((NCORES, P, SUMC2 * P), dtype=NPBF)
    S2[e_core2, e_p2, chunk2 * P + d_loc2] = w2e.astype(NPBF)

    # int16 gather indices: stream position i -> [col i//16, partition i%16],
    # replicated across the 8 16-partition groups. Padding points at row 0.
    idx2 = np.zeros((NCORES, P, SUMC2 * 8), dtype=np.int16)
    i2 = np.where(h2s == 0, r2, r2 - HALF).astype(np.int16)
    icol = chunk2 * 8 + (rank2 % P) // 16
    ipart = rank2 % 16
    for g in range(8):
        idx2[e_core2, 16 * g + ipart, icol] = i2

    meta = dict(N=N, E=E, DIN=DIN, HID=HID, OUT=OUT, B=B, NLOC=NLOC,
                TILES=TILES, HALF=HALF, GN=GN, GPC=GPC,
                C1=[int(c) for c in C1], SUMC1=SUMC1, C1MAX=C1MAX,
                C2=[(int(a), int(b)) for a, b in C2], SUMC2=SUMC2,
                C2TOTMAX=C2TOTMAX,
                ln_trivial=bool(np.all(np.asarray(ln_gamma) == 1.0)
                                and np.all(np.asarray(ln_beta) == 0.0)))

    W1T = np.asarray(W1, np.float32).T          # [DIN, HID]
    WresT = np.asarray(Wres, np.float32).T      # [DIN, HID]
    WclsT = np.asarray(Wcls, np.float32).T      # [2*HID, OUT]
    shared = dict(
        w1t_lo=np.ascontiguousarray(W1T[:P, :].astype(NPBF)),
        w1t_hi=np.ascontiguousarray(W1T[P:, :].astype(NPBF)),
        w2t=np.ascontiguousarray(np.asarray(W2, np.float32).T.astype(NPBF)),
        wrest_lo=np.ascontiguousarray(WresT[:P, :].astype(NPBF)),
        wrest_hi=np.ascontiguousarray(WresT[P:, :].astype(NPBF)),
        wcls_mean=np.ascontiguousarray((WclsT[:HID, :] / GN).astype(NPBF)),
        wcls_max=np.ascontiguousarray(WclsT[HID:, :].astype(NPBF)),
        bcls=np.ascontiguousarray(np.asarray(b_cls, np.float32)[:, None]),
        gam=np.ascontiguousarray(
            np.asarray(ln_gamma, np.float32)[:, None]),
        bet=np.ascontiguousarray(np.asarray(ln_beta, np.float32)[:, None]),
    )
    percore = []
    for c in range(NCORES):
        pg = perm[c * NLOC:(c + 1) * NLOC]
        percore.append(dict(
            g1=np.ascontiguousarray(G1[c]),
            s1=np.ascontiguousarray(S1[c]),
            s2=np.ascontiguousarray(S2[c]),
            idx2=np.ascontiguousarray(idx2[c]),
            xt_own=np.ascontiguousarray(
                np.asarray(X, np.float64)[pg].T.astype(NPBF)),
        ))
    return meta, shared, percore


# ------------------------------------------------------------- device program
def _build(meta):
    M = meta
    TILES, SUMC1, SUMC2 = M["TILES"], M["SUMC1"], M["SUMC2"]
    C1, C2 = M["C1"], M["C2"]
    C1MAX, C2TOTMAX = M["C1MAX"], M["C2TOTMAX"]
    DIN, HID, OUT = M["DIN"], M["HID"], M["OUT"]
    NLOC, HALF = M["NLOC"], M["HALF"]
    GN, GPC = M["GN"], M["GPC"]
    N = M["N"]
    cum1 = [0]
    for c in C1:
        cum1.append(cum1[-1] + c)
    cum2 = [0]
    for a, b in C2:
        cum2.append(cum2[-1] + a + b)

    nc = bacc.Bacc(num_devices=NCORES)

    g1_d = nc.dram_tensor("g1", [P, SUMC1 * DIN], BF16, kind="ExternalInput")
    s1_d = nc.dram_tensor("s1", [P, SUMC1 * P], BF16, kind="ExternalInput")
    s2_d = nc.dram_tensor("s2", [P, SUMC2 * P], BF16, kind="ExternalInput")
    idx2_d = nc.dram_tensor("idx2", [P, SUMC2 * 8], I16, kind="ExternalInput")
    xt_own_d = nc.dram_tensor("xt_own", [DIN, NLOC], BF16,
                              kind="ExternalInput")
    w1t_lo_d = nc.dram_tensor("w1t_lo", [P, HID], BF16, kind="ExternalInput")
    w1t_hi_d = nc.dram_tensor("w1t_hi", [P, HID], BF16, kind="ExternalInput")
    w2t_d = nc.dram_tensor("w2t", [HID, HID], BF16, kind="ExternalInput")
    wrest_lo_d = nc.dram_tensor("wrest_lo", [P, HID], BF16,
                                kind="ExternalInput")
    wrest_hi_d = nc.dram_tensor("wrest_hi", [P, HID], BF16,
                                kind="ExternalInput")
    wcls_mean_d = nc.dram_tensor("wcls_mean", [HID, OUT], BF16,
                                 kind="ExternalInput")
    wcls_max_d = nc.dram_tensor("wcls_max", [HID, OUT], BF16,
                                kind="ExternalInput")
    bcls_d = nc.dram_tensor("bcls", [OUT, 1], F32, kind="ExternalInput")
    if not M["ln_trivial"]:
        gam_d = nc.dram_tensor("gam", [HID, 1], F32, kind="ExternalInput")
        bet_d = nc.dram_tensor("bet", [HID, 1], F32, kind="ExternalInput")
    out_d = nc.dram_tensor("logits_t", [OUT, GPC], F32, kind="ExternalOutput")

    y2own_d = nc.dram_tensor("y2own", [NLOC, HID], BF16)
    y2full_d = nc.dram_tensor("y2full", [N, HID], BF16, addr_space="Shared")

    with tile.TileContext(nc) as tc, ExitStack() as ctx:
        cpool = ctx.enter_context(tc.tile_pool(name="consts", bufs=1))
        g1pool = ctx.enter_context(tc.tile_pool(name="g1s", bufs=3))
        s1pool = ctx.enter_context(tc.tile_pool(name="s1s", bufs=3))
        g2pool = ctx.enter_context(tc.tile_pool(name="g2s", bufs=3))
        s2pool = ctx.enter_context(tc.tile_pool(name="s2s", bufs=3))
        spool = ctx.enter_context(tc.tile_pool(name="small", bufs=4))
        # PSUM budget is 8 banks of [128 x 2KB]:
        #   agg: 4 slots (L1 aggLo/aggHi x2 tiles; L2 agg2/mu128/rstd128)
        #   mm : 2 slots (h1ps/y2ps/xres/stats/logits)  tr: 2 slots
        apool = ctx.enter_context(tc.tile_pool(name="aggp", bufs=4,
                                               space="PSUM"))
        mpool = ctx.enter_context(tc.tile_pool(name="mmp", bufs=2,
                                               space="PSUM"))
        tpool = ctx.enter_context(tc.tile_pool(name="trp", bufs=2,
                                               space="PSUM"))
        blk = ctx.enter_context(tc.tile_pool(name="blocks", bufs=1))

        # ---- constants
        ident = cpool.tile([P, P], BF16, tag="ident")
        make_identity(nc, ident[:])
        ones_col = cpool.tile([P, 1], F32, tag="ones_col")
        nc.vector.memset(ones_col[:], 1.0)
        ones_row = cpool.tile([1, P], F32, tag="ones_row")
        nc.vector.memset(ones_row[:], 1.0)
        eps_sb = cpool.tile([1, 1], F32, tag="eps")
        nc.vector.memset(eps_sb[:], 1e-5)

        w1t_lo = cpool.tile([P, HID], BF16, tag="w1t_lo")
        nc.sync.dma_start(w1t_lo[:], w1t_lo_d[:])
        w1t_hi = cpool.tile([P, HID], BF16, tag="w1t_hi")
        nc.sync.dma_start(w1t_hi[:], w1t_hi_d[:])
        w2t = cpool.tile([HID, HID], BF16, tag="w2t")
        nc.sync.dma_start(w2t[:], w2t_d[:])
        wrest_lo = cpool.tile([P, HID], BF16, tag="wrest_lo")
        nc.sync.dma_start(wrest_lo[:], wrest_lo_d[:])
        wrest_hi = cpool.tile([P, HID], BF16, tag="wrest_hi")
        nc.sync.dma_start(wrest_hi[:], wrest_hi_d[:])
        wcls_mean = cpool.tile([HID, OUT], BF16, tag="wcls_mean")
        nc.sync.dma_start(wcls_mean[:], wcls_mean_d[:])
        wcls_max = cpool.tile([HID, OUT], BF16, tag="wcls_max")
        nc.sync.dma_start(wcls_max[:], wcls_max_d[:])
        bcls = cpool.tile([OUT, 1], F32, tag="bcls")
        nc.sync.dma_start(bcls[:], bcls_d[:])
        if not M["ln_trivial"]:
            gam = cpool.tile([HID, 1], F32, tag="gam")
            nc.sync.dma_start(gam[:], gam_d[:])
            bet = cpool.tile([HID, 1], F32, tag="bet")
            nc.sync.dma_start(bet[:], bet_d[:])

        idx2_sb = cpool.tile([P, SUMC2 * 8], I16, tag="idx2")
        nc.sync.dma_start(idx2_sb[:], idx2_d[:])

        xresT = blk.tile([HID, NLOC], F32, tag="xresT")
        Hsum = blk.tile([HID, GPC], F32, tag="Hsum")
        nc.vector.memset(Hsum[:], 0.0)
        Hmax = blk.tile([HID, GPC], F32, tag="Hmax")
        nc.vector.memset(Hmax[:], -3.0e38)

        # ---- layer 1 + Y2 per dest tile
        for t in range(TILES):
            C = C1[t]
            b = cum1[t]
            g1 = g1pool.tile([P, C1MAX * DIN], BF16, tag="g1", name="g1t")
            nc.sync.dma_start(g1[:, :C * DIN],
                              g1_d[:, b * DIN:(b + C) * DIN])
            s1 = s1pool.tile([P, C1MAX * P], BF16, tag="s1", name="s1t")
            nc.sync.dma_start(s1[:, :C * P], s1_d[:, b * P:(b + C) * P])

            aggL = apool.tile([P, P], F32, tag="agg", name="aggL")
            aggH = apool.tile([P, P], F32, tag="agg", name="aggH")
            for c in range(C):
                sc = s1[:, c * P:(c + 1) * P]
                nc.tensor.matmul(aggL[:], lhsT=g1[:, c * DIN:c * DIN + P],
                                 rhs=sc, start=(c == 0), stop=(c == C - 1))
                nc.tensor.matmul(aggH[:],
                                 lhsT=g1[:, c * DIN + P:(c + 1) * DIN],
                                 rhs=sc, start=(c == 0), stop=(c == C - 1))
            aggL_sb = spool.tile([P, P], BF16, tag="aggL_sb")
            nc.vector.tensor_copy(aggL_sb[:], aggL[:])
            aggH_sb = spool.tile([P, P], BF16, tag="aggH_sb")
            nc.vector.tensor_copy(aggH_sb[:], aggH[:])
            h1ps = mpool.tile([HID, P], F32, tag="mm", name="h1ps")
            nc.tensor.matmul(h1ps[:], lhsT=w1t_lo[:], rhs=aggL_sb[:],
                             start=True, stop=False)
            nc.tensor.matmul(h1ps[:], lhsT=w1t_hi[:], rhs=aggH_sb[:],
                             start=False, stop=True)
            # R = relu(h1T) = dinv_d * h1  (dinv^2 folded into S1)
            R = spool.tile([HID, P], BF16, tag="R")
            nc.scalar.activation(R[:], h1ps[:], AF.Relu)
            y2ps = mpool.tile([HID, P], F32, tag="mm", name="y2ps")
            nc.tensor.matmul(y2ps[:], lhsT=w2t[:], rhs=R[:],
                             start=True, stop=True)
            y2sb = spool.tile([HID, P], BF16, tag="y2sb")
            nc.vector.tensor_copy(y2sb[:], y2ps[:])
            tr = tpool.tile([P, HID], BF16, tag="tr", name="trps")
            nc.tensor.transpose(tr[:], y2sb[:], ident[:])
            y2row = spool.tile([P, HID], BF16, tag="y2row")
            nc.vector.tensor_copy(y2row[:], tr[:])
            nc.sync.dma_start(y2own_d[t * P:(t + 1) * P, :], y2row[:])

        nc.gpsimd.collective_compute(
            "AllGather", ALU.bypass,
            replica_groups=[list(range(NCORES))],
            ins=[y2own_d[:]], outs=[y2full_d[:]])

        # ---- XresT = (X @ Wres.T).T per tile, kept resident (fills the
        # AllGather stall)
        for t in range(TILES):
            xa = spool.tile([P, P], BF16, tag="xa")
            nc.sync.dma_start(xa[:], xt_own_d[:P, t * P:(t + 1) * P])
            xb = spool.tile([P, P], BF16, tag="xb")
            nc.sync.dma_start(xb[:], xt_own_d[P:, t * P:(t + 1) * P])
            xps = mpool.tile([HID, P], F32, tag="mm", name="xps")
            nc.tensor.matmul(xps[:], lhsT=wrest_lo[:], rhs=xa[:],
                             start=True, stop=False)
            nc.tensor.matmul(xps[:], lhsT=wrest_hi[:], rhs=xb[:],
                             start=False, stop=True)
            nc.vector.tensor_copy(xresT[:, t * P:(t + 1) * P], xps[:])

        # ---- layer 2 per dest tile: gather Y2 rows, scatter-matmul, LN, pool
        for t in range(TILES):
            C0, C1h = C2[t]
            Ct = C0 + C1h
            cb = cum2[t]
            g2 = g2pool.tile([P, C2TOTMAX * HID], BF16, tag="g2", name="g2t")
            for half in range(2):
                Ch = C2[t][half]
                off = C0 if half else 0
                gv = g2[:, off * HID:(off + Ch) * HID].rearrange(
                    "p (c f) -> p c f", f=HID)
                nc.gpsimd.dma_gather(
                    gv, y2full_d[HALF:, :] if half else y2full_d[:HALF, :],
                    idx2_sb[:, (cb + off) * 8:(cb + off + Ch) * 8],
                    Ch * P, Ch * P, HID, single_packet=False)
            s2 = s2pool.tile([P, C2TOTMAX * P], BF16, tag="s2", name="s2t")
            nc.sync.dma_start(s2[:, :Ct * P],
                              s2_d[:, cb * P:(cb + Ct) * P])

            agg2 = apool.tile([HID, P], F32, tag="agg", name="agg2")
            for c in range(Ct):
                nc.tensor.matmul(agg2[:], lhsT=g2[:, c * HID:(c + 1) * HID],
                                 rhs=s2[:, c * P:(c + 1) * P],
                                 start=(c == 0), stop=(c == Ct - 1))
            h2 = spool.tile([HID, P], F32, tag="h2")
            nc.scalar.activation(h2[:], agg2[:], AF.Relu)
            nc.vector.tensor_tensor(out=h2[:], in0=h2[:],
                                    in1=xresT[:, t * P:(t + 1) * P],
                                    op=ALU.add)
            # LayerNorm over the feature (partition) axis, stats via matmul
            sq = spool.tile([HID, P], F32, tag="sq")
            nc.vector.tensor_tensor(out=sq[:], in0=h2[:], in1=h2[:],
                                    op=ALU.mult)
            stats_s = mpool.tile([1, P], F32, tag="mm", name="stats_s")
            nc.tensor.matmul(stats_s[:], lhsT=ones_col[:], rhs=h2[:],
                             start=True, stop=True)
            stats_q = mpool.tile([1, P], F32, tag="mm", name="stats_q")
            nc.tensor.matmul(stats_q[:], lhsT=ones_col[:], rhs=sq[:],
                             start=True, stop=True)
            # mu = s1/HID ; var = s2/HID - mu^2 ; rstd = 1/sqrt(var+eps)
            mu = spool.tile([1, P], F32, tag="murow")
            nc.vector.tensor_scalar_mul(mu[:], stats_s[:], 1.0 / HID)
            musq = spool.tile([1, P], F32, tag="musq")
            nc.vector.tensor_tensor(out=musq[:], in0=mu[:], in1=mu[:],
                                    op=ALU.mult)
            var = spool.tile([1, P], F32, tag="var")
            nc.vector.tensor_scalar(out=var[:], in0=stats_q[:],
                                    scalar1=1.0 / HID, scalar2=None,
                                    op0=ALU.mult)
            nc.vector.tensor_tensor(out=var[:], in0=var[:], in1=musq[:],
                                    op=ALU.subtract)
            std = spool.tile([1, P], F32, tag="std")
            nc.scalar.activation(std[:], var[:], AF.Sqrt,
                                 bias=eps_sb[:], scale=1.0)
            rstd = spool.tile([1, P], F32, tag="rstd")
            nc.vector.reciprocal(rstd[:], std[:])
            mu128 = apool.tile([HID, P], F32, tag="agg", name="mu128")
            nc.tensor.matmul(mu128[:], lhsT=ones_row[:], rhs=mu[:],
                             start=True, stop=True)
            rstd128 = apool.tile([HID, P], F32, tag="agg", name="rstd128")
            nc.tensor.matmul(rstd128[:], lhsT=ones_row[:], rhs=rstd[:],
                             start=True, stop=True)
            nc.vector.tensor_tensor(out=h2[:], in0=h2[:], in1=mu128[:],
                                    op=ALU.subtract)
            nc.vector.tensor_tensor(out=h2[:], in0=h2[:], in1=rstd128[:],
                                    op=ALU.mult)
            if not M["ln_trivial"]:
                nc.vector.tensor_scalar(out=h2[:], in0=h2[:],
                                        scalar1=gam[:], scalar2=None,
                                        op0=ALU.mult)
                nc.vector.tensor_scalar(out=h2[:], in0=h2[:],
                                        scalar1=bet[:], scalar2=None,
                                        op0=ALU.add)
            # incremental pooling: node cols [t*P, (t+1)*P) -> graphs
            lo = t * P
            while lo < (t + 1) * P:
                g_ = lo // GN
                hi = min((g_ + 1) * GN, (t + 1) * P)
                a, b2 = lo - t * P, hi - t * P
                psum = spool.tile([HID, 1], F32, tag="psum")
                nc.vector.tensor_reduce(psum[:], h2[:, a:b2], axis=AX.X,
                                        op=ALU.add)
                nc.vector.tensor_tensor(out=Hsum[:, g_:g_ + 1],
                                        in0=Hsum[:, g_:g_ + 1], in1=psum[:],
                                        op=ALU.add)
                pmax = spool.tile([HID, 1], F32, tag="pmax")
                nc.vector.tensor_reduce(pmax[:], h2[:, a:b2], axis=AX.X,
                                        op=ALU.max)
                nc.vector.tensor_tensor(out=Hmax[:, g_:g_ + 1],
                                        in0=Hmax[:, g_:g_ + 1], in1=pmax[:],
                                        op=ALU.max)
                lo = hi

        # ---- classifier: logits.T = WclsT_mean.T @ (Hsum/GN) + ...
        hsum_bf = spool.tile([HID, GPC], BF16, tag="hsum_bf")
        nc.vector.tensor_copy(hsum_bf[:], Hsum[:])
        hmax_bf = spool.tile([HID, GPC], BF16, tag="hmax_bf")
        nc.vector.tensor_copy(hmax_bf[:], Hmax[:])
        lg = mpool.tile([OUT, GPC], F32, tag="mm", name="lg")
        nc.tensor.matmul(lg[:], lhsT=wcls_mean[:], rhs=hsum_bf[:],
                         start=True, stop=False)
        nc.tensor.matmul(lg[:], lhsT=wcls_max[:], rhs=hmax_bf[:],
                         start=False, stop=True)
        osb = spool.tile([OUT, GPC], F32, tag="osb")
        nc.vector.tensor_copy(osb[:], lg[:])
        nc.vector.tensor_scalar_add(osb[:], osb[:], bcls[:])
        nc.sync.dma_start(out_d[:], osb[:])

    nc.compile()
    return nc


def _make_in_maps(meta, shared, percore):
    in_maps = []
    for c in range(NCORES):
        m = dict(shared)
        if meta["ln_trivial"]:
            m.pop("gam"), m.pop("bet")
        m.update(percore[c])
        in_maps.append(m)
    return in_maps


_CACHE = {}


def kernel(**inputs):
    meta, shared, percore = _prep(**inputs)
    key = (meta["N"], meta["E"], meta["DIN"], meta["HID"], meta["OUT"],
           meta["B"], tuple(meta["C1"]), tuple(meta["C2"]),
           meta["ln_trivial"])
    if key not in _CACHE:
        _CACHE[key] = _build(meta)
    nc = _CACHE[key]

    in_maps = _make_in_maps(meta, shared, percore)
    res = run_bass_kernel_spmd(nc, in_maps, list(range(NCORES)))
    outs = [np.asarray(res.results[c]["logits_t"]).T for c in range(NCORES)]
    return np.ascontiguousarray(np.concatenate(outs, axis=0), dtype=np.float32)
